# revision 1
# baseline (speedup 1.0000x reference)
"""GCN layer on trn2: out = relu(A_sparse @ (x @ W) + b).

Strategy (8-core SPMD, dest-node row partition):
  - Reorder as (A @ x) @ W  (valid since F_in == F_out matmul associativity).
  - Shard destination nodes across 8 cores (12500 nodes/core); each core
    handles the contiguous slice of (row-sorted) edges whose dest is in its
    shard.
  - Per core: edges are gathered from the full x table in HBM with
    dma_gather (128 rows of 512B per tile), the segment-sum over sorted
    dest rows is done with a one-hot "selection" matmul into PSUM
    (S_T[e, d] = val[e] * (dest_rel[e] == d), G.T[f, d] += msgs.T @ S_T),
    then per 256-dest superblock: G.T @ W + b, relu, store.
  - dma_gather indices are int16, so the x table is split into 4 chunks of
    25000 rows; edges are grouped by (superblock, chunk) and padded to
    multiples of 128 with zero-val edges. Group sizes are padded up to the
    max across all 8 cores so a single compiled program serves all cores
    (SPMD).
"""

import sys

for _p in ("/opt/trn_rl_repo",):
    if _p not in sys.path:
        sys.path.insert(0, _p)

import numpy as np
from dataclasses import dataclass

import concourse.bacc as bacc
import concourse.bass as bass
import concourse.mybir as mybir
import concourse.tile as tile
from concourse.bass_utils import run_bass_kernel_spmd


def _cdiv(a, b):
    return (a + b - 1) // b


@dataclass
class Cfg:
    n_nodes: int = 100000
    f: int = 128            # feature dim (in == out == 128)
    n_cores: int = 8
    sb: int = 256           # dest nodes per one-hot matmul / PSUM tile
    sg_size: int = 4        # superblocks per supergroup (PSUM group)
    n_ch: int = 4           # x-table chunks (dma_gather int16 index limit)
    gather_dt: str = "float32"   # float32 | float16 | bfloat16
    use_f32r: bool = True   # bitcast f32 matmul operands to float32r
    gbuf_bufs: int = 5
    st_bufs: int = 6
    repeat: int = 1
    n_queues: int = 4
    max_gather_tiles: int = 16
    dma_scratch: int = 16384
    mode: str = "full"   # full | gather | nohot  (ablation-only)
    pgt_bufs: int = 5
    pout_bufs: int = 3

    @property
    def npc(self):
        return self.n_nodes // self.n_cores

    @property
    def ch_rows(self):
        return self.n_nodes // self.n_ch

    @property
    def n_sb(self):
        return _cdiv(self.npc, self.sb)

    @property
    def n_sg(self):
        return _cdiv(self.n_sb, self.sg_size)

    @property
    def np_gdt(self):
        import ml_dtypes
        return {
            "float32": np.float32,
            "float16": np.float16,
            "bfloat16": ml_dtypes.bfloat16,
        }[self.gather_dt]

    @property
    def bir_gdt(self):
        return {
            "float32": mybir.dt.float32,
            "float16": mybir.dt.float16,
            "bfloat16": mybir.dt.bfloat16,
        }[self.gather_dt]


def _groups(cfg):
    """(sb, ch) pairs in global iteration order: sg -> ch -> sb."""
    out = []
    for sg in range(cfg.n_sg):
        sbs = range(sg * cfg.sg_size, min((sg + 1) * cfg.sg_size, cfg.n_sb))
        for ch in range(cfg.n_ch):
            for sb in sbs:
                out.append((sb, ch))
    return out


def _plan(cfg, T):
    """Per-supergroup tile plan. T: [n_sb, n_ch] tiles per group."""
    plans = []
    t_global = 0
    for sg in range(cfg.n_sg):
        sbs = list(range(sg * cfg.sg_size, min((sg + 1) * cfg.sg_size, cfg.n_sb)))
        t0 = t_global
        runs = []
        for ch in range(cfg.n_ch):
            entries = [(sb, int(T[sb, ch])) for sb in sbs if T[sb, ch] > 0]
            ntiles = sum(n for _, n in entries)
            runs.append((ch, entries, ntiles))
            t_global += ntiles
        plans.append(dict(sg=sg, sbs=sbs, t0=t0, runs=runs,
                          ntiles=t_global - t0))
    return plans, t_global


def _host_prep(cfg, adj_rows, adj_cols, adj_vals):
    """Partition/pad/order edges per core; build device metadata arrays."""
    rows = np.ascontiguousarray(adj_rows).astype(np.int64)
    cols = np.ascontiguousarray(adj_cols).astype(np.int64)
    vals = np.ascontiguousarray(adj_vals).astype(np.float64)
    gnp = cfg.np_gdt

    bounds = np.searchsorted(rows, np.arange(cfg.n_cores + 1) * cfg.npc)
    counts = np.zeros((cfg.n_cores, cfg.n_sb, cfg.n_ch), np.int64)
    core_edges = []
    for s in range(cfg.n_cores):
        e0, e1 = int(bounds[s]), int(bounds[s + 1])
        r = rows[e0:e1] - s * cfg.npc
        c = cols[e0:e1]
        v = vals[e0:e1]
        sb = r // cfg.sb
        ch = c // cfg.ch_rows
        sg = sb // cfg.sg_size
        key = (sg * cfg.n_ch + ch) * cfg.sg_size + (sb - sg * cfg.sg_size)
        order = np.argsort(key, kind="stable")
        core_edges.append((r[order], c[order], v[order], key[order]))
        np.add.at(counts[s], (sb, ch), 1)

    T = ((counts + 127) // 128).max(axis=0)  # [n_sb, n_ch] uniform tiles
    groups = _groups(cfg)
    NT = int(sum(T[sb, ch] for sb, ch in groups))

    metas = []
    for s in range(cfg.n_cores):
        r, c, v, key = core_edges[s]
        idxl = np.zeros(NT * 128, np.int16)
        rrel = np.zeros(NT * 128, np.float64)
        vv = np.zeros(NT * 128, np.float64)
        pos = 0
        for sb, ch in groups:
            t = int(T[sb, ch])
            if t == 0:
                continue
            sg = sb // cfg.sg_size
            kval = (sg * cfg.n_ch + ch) * cfg.sg_size + (sb - sg * cfg.sg_size)
            g0 = np.searchsorted(key, kval, "left")
            g1 = np.searchsorted(key, kval, "right")
            n = g1 - g0
            sl = slice(pos * 128, pos * 128 + n)
            idxl[sl] = (c[g0:g1] - ch * cfg.ch_rows).astype(np.int16)
            rrel[sl] = r[g0:g1] - sb * cfg.sb
            vv[sl] = v[g0:g1]
            pos += t
        assert pos == NT
        metas.append(dict(
            idx16=np.ascontiguousarray(np.tile(idxl.reshape(-1, 16).T, (8, 1))),
            rowrel=np.ascontiguousarray(rrel.reshape(NT, 128).T.astype(np.float32)),
            vals=np.ascontiguousarray(vv.reshape(NT, 128).T.astype(np.float32)),
        ))
    return T, NT, metas


def _build(cfg, T, NT):
    gdt = cfg.bir_gdt
    f32 = mybir.dt.float32
    F = cfg.f
    Relu = mybir.ActivationFunctionType.Relu
    Copy = mybir.ActivationFunctionType.Copy

    nc = bacc.Bacc("TRN2", target_bir_lowering=False, debug=False,
                   enable_asserts=False, num_devices=cfg.n_cores,
                   num_swdge_queues=cfg.n_queues,
                   dynamic_dma_scratch_size=cfg.dma_scratch)

    f32r_mode = cfg.use_f32r and gdt == f32
    xt_dt = mybir.dt.float32r if f32r_mode else gdt
    xt = nc.dram_tensor("xt", [cfg.n_nodes, F], xt_dt, kind="ExternalInput")
    idx16 = nc.dram_tensor("idx16", [128, NT * 8], mybir.dt.int16,
                           kind="ExternalInput")
    rowrel = nc.dram_tensor("rowrel", [128, NT], f32, kind="ExternalInput")
    valsd = nc.dram_tensor("vals", [128, NT], f32, kind="ExternalInput")
    iota = nc.dram_tensor("iota", [128, cfg.sb], gdt, kind="ExternalInput")
    wmat = nc.dram_tensor("wmat", [F, F], f32, kind="ExternalInput")
    biasb = nc.dram_tensor("biasb", [128, F], f32, kind="ExternalInput")
    outd = nc.dram_tensor("out", [cfg.npc, F], f32, kind="ExternalOutput")

    plans, nt2 = _plan(cfg, T)
    assert nt2 == NT
    max_run = max((ntiles for p in plans for (_, _, ntiles) in p["runs"]),
                  default=1)
    max_sgt = max((p["ntiles"] for p in plans), default=1)

    with tile.TileContext(nc) as tc:
        with (
            tc.tile_pool(name="const", bufs=1) as constp,
            tc.tile_pool(name="meta", bufs=2) as metap,
            tc.tile_pool(name="gbuf", bufs=cfg.gbuf_bufs) as gbufp,
            tc.tile_pool(name="stp", bufs=cfg.st_bufs) as stp,
            tc.tile_pool(name="eplg", bufs=3) as eplgp,
            tc.tile_pool(name="outs", bufs=4) as outsp,
            tc.tile_pool(name="pgt", bufs=cfg.pgt_bufs, space="PSUM") as pgt,
            tc.tile_pool(name="pout", bufs=cfg.pout_bufs, space="PSUM") as pout,
        ):
            iota_s = constp.tile([128, cfg.sb], gdt, tag="iota")
            nc.sync.dma_start(out=iota_s[:], in_=iota[:, :])
            w_s = constp.tile([F, F], f32, tag="w")
            nc.sync.dma_start(out=w_s[:], in_=wmat[:, :])
            bias_s = constp.tile([128, F], f32, tag="bias")
            nc.sync.dma_start(out=bias_s[:], in_=biasb[:, :])

            g_ctr = [0]
            for _rep in range(cfg.repeat):
              for plan in plans:
                sgt = plan["ntiles"]
                t0 = plan["t0"]
                sb_total = {sb: 0 for sb in plan["sbs"]}
                for (_, entries, _) in plan["runs"]:
                    for sb, n in entries:
                        sb_total[sb] += n

                if sgt > 0:
                    rr_s = metap.tile([128, max_sgt], f32, tag="rr")
                    nc.sync.dma_start(out=rr_s[:, :sgt],
                                      in_=rowrel[:, t0:t0 + sgt])
                    vv_s = metap.tile([128, max_sgt], f32, tag="vv")
                    nc.sync.dma_start(out=vv_s[:, :sgt],
                                      in_=valsd[:, t0:t0 + sgt])
                    ix_s = metap.tile([128, max_sgt * 8], mybir.dt.int16,
                                      tag="ix")
                    nc.sync.dma_start(out=ix_s[:, :sgt * 8],
                                      in_=idx16[:, t0 * 8:(t0 + sgt) * 8])

                gt_tiles = {}
                for sb in plan["sbs"]:
                    if sb_total[sb] > 0:
                        gt_tiles[sb] = pgt.tile([128, cfg.sb], f32, tag="gt",
                                                name="gt")

                sb_seen = {sb: 0 for sb in plan["sbs"]}
                tloc = 0
                mgt = cfg.max_gather_tiles
                for (ch, entries, ntiles) in plan["runs"]:
                    if ntiles == 0:
                        continue
                    gb = gbufp.tile([128, max_run, F], xt_dt, tag="gb")
                    for g0 in range(0, ntiles, mgt):
                        gn = min(mgt, ntiles - g0)
                        nc.gpsimd.dma_gather(
                            gb[:, g0:g0 + gn, :],
                            xt[ch * cfg.ch_rows:(ch + 1) * cfg.ch_rows, :],
                            ix_s[:, (tloc + g0) * 8:(tloc + g0 + gn) * 8],
                            gn * 128,
                            gn * 128,
                            F,
                            single_packet=False,
                            queue_num=g_ctr[0] % cfg.n_queues,
                        )
                        g_ctr[0] += 1
                    if cfg.mode == "gather":
                        sink = stp.tile([128, 128], gdt, tag="sink",
                                        name="sink")
                        nc.vector.tensor_copy(out=sink[:], in_=gb[:, 0, :])
                        tloc += ntiles
                        continue
                    st_shared = None
                    if cfg.mode == "nohot":
                        st_dt = mybir.dt.float32r if f32r_mode else gdt
                        st_shared = stp.tile([128, cfg.sb], st_dt, tag="st",
                                             name="st_shared")
                        nc.vector.tensor_scalar(
                            out=st_shared[:],
                            in0=iota_s[:],
                            scalar1=rr_s[:, tloc:tloc + 1],
                            scalar2=vv_s[:, tloc:tloc + 1],
                            op0=mybir.AluOpType.is_equal,
                            op1=mybir.AluOpType.mult,
                        )
                    rloc = tloc
                    for sb, ntl in entries:
                        for _k in range(ntl):
                            if st_shared is not None:
                                st = st_shared
                            else:
                                st_dt = (mybir.dt.float32r if f32r_mode
                                         else gdt)
                                st = stp.tile([128, cfg.sb], st_dt, tag="st")
                                nc.vector.tensor_scalar(
                                    out=st[:],
                                    in0=iota_s[:],
                                    scalar1=rr_s[:, rloc:rloc + 1],
                                    scalar2=vv_s[:, rloc:rloc + 1],
                                    op0=mybir.AluOpType.is_equal,
                                    op1=mybir.AluOpType.mult,
                                )
                            lhsT = gb[:, rloc - tloc, :]
                            rhs = st[:]
                            nc.tensor.matmul(
                                out=gt_tiles[sb][:],
                                lhsT=lhsT,
                                rhs=rhs,
                                start=(sb_seen[sb] == 0),
                                stop=(sb_seen[sb] == sb_total[sb] - 1),
                            )
                            sb_seen[sb] += 1
                            rloc += 1
                    tloc += ntiles

                # epilogue: per superblock apply W, bias, relu, store
                for sb in (plan["sbs"] if cfg.mode != "gather" else []):
                    gts = eplgp.tile([128, cfg.sb], f32, tag="gts")
                    if sb_total[sb] > 0:
                        nc.scalar.activation(out=gts[:], in_=gt_tiles[sb][:],
                                             func=Copy)
                    else:
                        nc.vector.memset(gts[:], 0.0)
                    for h in range(cfg.sb // 128):
                        row0 = sb * cfg.sb + h * 128
                        nrows = min(128, cfg.npc - row0)
                        if nrows <= 0:
                            break
                        po = pout.tile([128, F], f32, tag="po")
                        nc.tensor.matmul(out=po[:],
                                         lhsT=gts[:, h * 128:(h + 1) * 128],
                                         rhs=w_s[:], start=True, stop=True)
                        tmp = eplgp.tile([128, F], f32, tag="tmp")
                        nc.vector.tensor_tensor(out=tmp[:], in0=po[:],
                                                in1=bias_s[:],
                                                op=mybir.AluOpType.add)
                        ot = outsp.tile([128, F], f32, tag="ot")
                        nc.scalar.activation(out=ot[:], in_=tmp[:], func=Relu)
                        nc.sync.dma_start(out=outd[row0:row0 + nrows, :],
                                          in_=ot[:nrows, :])
    nc.compile()
    return nc


def _make_inputs(cfg, x, W, b, metas):
    gnp = cfg.np_gdt
    xt = np.ascontiguousarray(x).astype(gnp)
    iota = np.tile(np.arange(cfg.sb, dtype=np.float64), (128, 1)).astype(gnp)
    wmat = np.ascontiguousarray(W).astype(np.float32)
    biasb = np.tile(np.asarray(b, np.float32)[None, :], (128, 1))
    biasb = np.ascontiguousarray(biasb)
    in_maps = []
    for s in range(cfg.n_cores):
        m = metas[s]
        in_maps.append({
            "xt": xt,
            "idx16": m["idx16"],
            "rowrel": m["rowrel"],
            "vals": m["vals"],
            "iota": iota,
            "wmat": wmat,
            "biasb": biasb,
        })
    return in_maps


_BUILD_CACHE = {}


def _get_built(cfg, adj_rows, adj_cols, adj_vals):
    T, NT, metas = _host_prep(cfg, adj_rows, adj_cols, adj_vals)
    key = (cfg.gather_dt, cfg.use_f32r, cfg.sb, cfg.sg_size, NT,
           T.tobytes())
    if key not in _BUILD_CACHE:
        _BUILD_CACHE[key] = _build(cfg, T, NT)
    return _BUILD_CACHE[key], metas


def kernel(x, adj_rows, adj_cols, adj_vals, W, b):
    cfg = Cfg()
    nc, metas = _get_built(cfg, adj_rows, adj_cols, adj_vals)
    in_maps = _make_inputs(cfg, x, W, b, metas)
    res = run_bass_kernel_spmd(nc, in_maps, list(range(cfg.n_cores)))
    out = np.concatenate(
        [res.results[s]["out"] for s in range(cfg.n_cores)], axis=0)
    return out.astype(np.float32)

